# revision 14
# baseline (speedup 1.0000x reference)
"""Trainium2 Bass kernel for nn_Attention_26628797235287.

GQA attention layer (B=2, S=2048, HID=2048, 16 q-heads, 2 kv-heads, HD=128)
with RoPE, causal mask, softmax, and output projection. Returns
(out [B,S,HID], attn_weights [B,NH,S,S]) like the reference.

Sharding: tensor-parallel over heads across 8 cores. Core c computes q-heads
{2c, 2c+1} and kv-head c//4. Each core produces its 2 heads' attn_weights
slab and a partial output projection (host reduces the 8 partials).

Device dataflow (per core), all matmuls in float32r (fp32 with 11-bit
mantissa RNE rounding - full PE rate at moving dim >= 256):
  - host pre-transposes hidden/weights/RoPE tables so all device matmuls
    contract over the partition dim with no on-chip layout changes
  - QKV projections accumulate in PSUM over 16 HID-chunks; ACT epilogue
    adds biases; DVE applies RoPE in [d, token] layout
  - scores S = Q^T.T K^T per 128-row q-block accumulate in PSUM; causal
    masking adds a -1e9 upper-triangle constant to the diagonal block only
    (off-diagonal upper blocks are never computed; the attn_weights output
    buffer is pre-zeroed by the runtime)
  - softmax: ACT exp(scale*s) with fused row-sum accumulator, DVE
    reciprocal + in-place normalize (no max subtraction: |s*scale| is
    bounded ~ |q||k|*scale, far below exp overflow)
  - P rows transpose via PE into a [k, q] staging buffer; A@V accumulates
    O^T = sum_k V_k.T P^T_k with 512-wide moving operand
  - output projection contracts O^T against host-pre-transposed Wo slice
"""

import math

import numpy as np

B = 2
S = 2048
HID = 2048
NH = 16
NKV = 2
HD = 128
N_REP = NH // NKV
SCALE = HD ** -0.5
NEG = -1e9
T = B * S
HQ = NH // 8  # q heads per core
N_CORES = 8

_CACHE = {}


def _build_nc(causal: bool, s: int = S, b: int = B, hid: int = HID):
    import concourse.tile as tile
    from concourse import bacc, mybir
    from concourse.masks import make_causal_mask, make_identity

    f32 = mybir.dt.float32
    f32r = mybir.dt.float32r

    t = b * s
    n_hc = hid // 128       # HID chunks (contraction)
    n_tsb = t // 512        # token superblocks
    n_qb = s // 128         # q blocks per batch
    qsb_sz = min(512, s)    # q superblock
    n_qsb = s // qsb_sz
    qb_per_sb = qsb_sz // 128
    n_kc = s // 128         # k chunks per batch

    nc = bacc.Bacc("TRN2", target_bir_lowering=False, debug=False)

    hidden_t = nc.dram_tensor("hidden_t", [hid, t], f32r, kind="ExternalInput").ap()
    cos_t = nc.dram_tensor("cos_t", [HD, t], f32, kind="ExternalInput").ap()
    sinneg_t = nc.dram_tensor("sinneg_t", [HD, t], f32, kind="ExternalInput").ap()
    wq_t = nc.dram_tensor("wq_t", [hid, HQ * HD], f32r, kind="ExternalInput").ap()
    wk_t = nc.dram_tensor("wk_t", [hid, HD], f32r, kind="ExternalInput").ap()
    wv_t = nc.dram_tensor("wv_t", [hid, HD], f32r, kind="ExternalInput").ap()
    wo_t = nc.dram_tensor("wo_t", [HQ * HD, hid], f32r, kind="ExternalInput").ap()
    bq = nc.dram_tensor("bq", [HD, HQ], f32, kind="ExternalInput").ap()
    bk = nc.dram_tensor("bk", [HD, 1], f32, kind="ExternalInput").ap()
    bv = nc.dram_tensor("bv", [HD, 1], f32, kind="ExternalInput").ap()
    if not causal:
        # mask already divided by SCALE on host
        masks = nc.dram_tensor("masks", [b, s, s], f32, kind="ExternalInput").ap()

    outp = nc.dram_tensor("outp", [t, hid], f32, kind="ExternalOutput").ap()
    attnw = nc.dram_tensor("attnw", [b, HQ, s, s], f32, kind="ExternalOutput").ap()

    with tile.TileContext(nc) as tc:
        with (
            tc.tile_pool(name="consts", bufs=1) as consts,
            tc.tile_pool(name="resid", bufs=1) as resid,
        ):
            # ---- constants / weights ----
            wq_sb = consts.tile([128, n_hc, HQ * HD], f32r)
            nc.sync.dma_start(out=wq_sb, in_=wq_t.rearrange("(k p) d -> p k d", p=128))
            wk_sb = consts.tile([128, n_hc, HD], f32r)
            nc.sync.dma_start(out=wk_sb, in_=wk_t.rearrange("(k p) d -> p k d", p=128))
            wv_sb = consts.tile([128, n_hc, HD], f32r)
            nc.sync.dma_start(out=wv_sb, in_=wv_t.rearrange("(k p) d -> p k d", p=128))
            wo_sb = consts.tile([128, HQ, hid], f32r)
            nc.sync.dma_start(out=wo_sb, in_=wo_t.rearrange("(h p) d -> p h d", p=128))
            bq_sb = consts.tile([128, HQ], f32)
            nc.sync.dma_start(out=bq_sb, in_=bq)
            bk_sb = consts.tile([128, 1], f32)
            nc.sync.dma_start(out=bk_sb, in_=bk)
            bv_sb = consts.tile([128, 1], f32)
            nc.sync.dma_start(out=bv_sb, in_=bv)
            ident_f = consts.tile([128, 128], f32)
            make_identity(nc, ident_f)
            ident = consts.tile([128, 128], f32r)
            nc.vector.tensor_copy(ident, ident_f)
            if causal:
                diag = consts.tile([128, 128], f32)
                make_causal_mask(nc, diag, mask_val=NEG)

            # ---- residents ----
            qr_sb = resid.tile([128, HQ, t], f32r)   # Q^T roped
            kr_sb = resid.tile([128, t], f32r)       # K^T roped
            v_sb = resid.tile([128, n_kc * b, HD], f32r)  # V token-major
            or_sb = resid.tile([128, HQ, t], f32r)   # O^T attention out

            # ---- phase B: QKV projections + RoPE + V transpose, per 512-token sb
            with (
                tc.tile_pool(name="hsb_pool", bufs=2) as hsb_pool,
                tc.tile_pool(name="pre_pool", bufs=1) as pre_pool,
                tc.tile_pool(name="tab_pool", bufs=1) as tab_pool,
                tc.tile_pool(name="tmp_pool", bufs=2) as tmp_pool,
                tc.tile_pool(name="qkv_ps", bufs=6, space="PSUM") as qkv_ps,
                tc.tile_pool(name="vtr_ps", bufs=2, space="PSUM") as vtr_ps,
            ):
                nhh = n_hc // 2  # hid chunks per half
                for isb in range(n_tsb):
                    tk = slice(isb * 512, (isb + 1) * 512)
                    pss = []
                    for tgt in range(HQ + 2):
                        pss.append(qkv_ps.tile([128, 512], mybir.dt.float32,
                                               name=f"qkvps{isb}_{tgt}", tag="qkvps"))
                    for half in range(2):
                        hsb = hsb_pool.tile([128, nhh, 512], f32r,
                                            name=f"hsb{isb}_{half}", tag="hsb")
                        nc.sync.dma_start(
                            out=hsb,
                            in_=hidden_t[half * (hid // 2):(half + 1) * (hid // 2), tk]
                            .rearrange("(k p) n -> p k n", p=128),
                        )
                        for tgt in range(HQ + 2):
                            if tgt < HQ:
                                w_ap = wq_sb[:, :, tgt * HD:(tgt + 1) * HD]
                            elif tgt == HQ:
                                w_ap = wk_sb
                            else:
                                w_ap = wv_sb
                            for kc in range(nhh):
                                kcg = half * nhh + kc
                                nc.tensor.matmul(
                                    pss[tgt], w_ap[:, kcg, :], hsb[:, kc, :],
                                    start=(kcg == 0), stop=(kcg == n_hc - 1),
                                )
                    # epilogues: bias add into pre-rope f32 tiles
                    pre = {}
                    for tgt in range(HQ + 2):
                        pt = pre_pool.tile([128, 512], f32,
                                           name=f"pre{isb}_{tgt}", tag=f"pre{tgt}")
                        bias = (bq_sb[:, tgt:tgt + 1] if tgt < HQ
                                else (bk_sb if tgt == HQ else bv_sb))
                        nc.scalar.activation(
                            pt, pss[tgt], mybir.ActivationFunctionType.Identity,
                            bias=bias,
                        )
                        pre[tgt] = pt
                    # RoPE for q heads and k
                    cos_l = tab_pool.tile([128, 512], f32, name=f"cos{isb}", tag="cosl")
                    sin_l = tab_pool.tile([128, 512], f32, name=f"sin{isb}", tag="sinl")
                    nc.sync.dma_start(out=cos_l, in_=cos_t[:, tk])
                    nc.sync.dma_start(out=sin_l, in_=sinneg_t[:, tk])
                    for tgt in range(HQ + 1):
                        src = pre[tgt]
                        dst = (qr_sb[:, tgt, tk] if tgt < HQ else kr_sb[:, tk])
                        trot = tmp_pool.tile([128, 512], f32,
                                             name=f"trot{isb}_{tgt}", tag="trot")
                        tcos = tmp_pool.tile([128, 512], f32,
                                             name=f"tcos{isb}_{tgt}", tag="tcos")
                        nc.vector.tensor_copy(trot[0:64, :], src[64:128, :])
                        nc.vector.tensor_copy(trot[64:128, :], src[0:64, :])
                        nc.vector.tensor_mul(trot, trot, sin_l)
                        nc.vector.tensor_mul(tcos, src, cos_l)
                        nc.vector.tensor_add(dst, trot, tcos)
                    # V: round + transpose to token-major
                    vt_r = tmp_pool.tile([128, 512], f32r, name=f"vtr{isb}", tag="vt_r")
                    nc.vector.tensor_copy(vt_r, pre[HQ + 1])
                    for j in range(4):
                        c = isb * 4 + j
                        vp = vtr_ps.tile([128, 128], f32r, name=f"vp{c}", tag="vp")
                        nc.tensor.transpose(vp, vt_r[:, j * 128:(j + 1) * 128], ident)
                        nc.vector.tensor_copy(v_sb[:, c, :], vp)

            # ---- phase D: attention ----
            with (
                tc.tile_pool(name="s_ps", bufs=2, space="PSUM") as s_ps,
                tc.tile_pool(name="pt_ps", bufs=2, space="PSUM") as pt_ps,
                tc.tile_pool(name="o_ps", bufs=2, space="PSUM") as o_ps,
                tc.tile_pool(name="ep_pool", bufs=3) as ep_pool,
                tc.tile_pool(name="small", bufs=8) as small,
                tc.tile_pool(name="stage_pool", bufs=1) as stage_pool,
                tc.tile_pool(name="mask_pool", bufs=3) as mask_pool,
            ):
                for ib in range(b):
                    for h in range(HQ):
                        for iqsb in range(n_qsb):
                            stage = stage_pool.tile(
                                [128, n_kc, qsb_sz], f32r,
                                name=f"stage{ib}_{h}_{iqsb}", tag="stage",
                            )
                            kx_sb = (iqsb + 1) * qsb_sz if causal else s
                            for iqb in range(qb_per_sb):
                                qi = iqsb * qb_per_sb + iqb
                                q0 = qi * 128
                                kx = (qi + 1) * 128 if causal else s
                                ep = ep_pool.tile([128, s], f32r,
                                                  name=f"ep{ib}{h}{qi}", tag="ep")
                                r_parts = []
                                for ih in range((kx + 1023) // 1024):
                                    c0 = ih * 1024
                                    cw = min(1024, kx - c0)
                                    ps = s_ps.tile([128, 1024], mybir.dt.float32,
                                                   name=f"sps{ib}{h}{qi}{ih}", tag="sps")
                                    nch = (cw + 511) // 512
                                    for c in range(nch):
                                        w = min(512, cw - c * 512)
                                        nc.tensor.matmul(
                                            ps[:, c * 512:c * 512 + w],
                                            qr_sb[:, h, ib * s + q0:ib * s + q0 + 128],
                                            kr_sb[:, ib * s + c0 + c * 512:
                                                  ib * s + c0 + c * 512 + w],
                                            start=True, stop=True,
                                        )
                                    if causal and c0 + cw == kx:
                                        nc.vector.tensor_add(
                                            ps[:, cw - 128:cw], ps[:, cw - 128:cw], diag
                                        )
                                    if not causal:
                                        mk = mask_pool.tile([128, 1024], f32,
                                                            name=f"mk{ib}{h}{qi}{ih}",
                                                            tag="mk")
                                        nc.sync.dma_start(
                                            out=mk[:, :cw],
                                            in_=masks[ib, q0:q0 + 128, c0:c0 + cw],
                                        )
                                        nc.vector.tensor_add(
                                            ps[:, :cw], ps[:, :cw], mk[:, :cw]
                                        )
                                    rp = small.tile([128, 1], f32,
                                                    name=f"rp{ib}{h}{qi}{ih}", tag="rp")
                                    nc.scalar.activation(
                                        ep[:, c0:c0 + cw], ps[:, :cw],
                                        mybir.ActivationFunctionType.Exp,
                                        scale=float(SCALE), accum_out=rp,
                                    )
                                    r_parts.append(rp)
                                rinv = small.tile([128, 1], f32,
                                                  name=f"rinv{ib}{h}{qi}", tag="rinv")
                                if len(r_parts) == 2:
                                    rsum = small.tile([128, 1], f32,
                                                      name=f"rs{ib}{h}{qi}", tag="rs")
                                    nc.vector.tensor_add(rsum, r_parts[0], r_parts[1])
                                    nc.vector.reciprocal(rinv, rsum)
                                else:
                                    nc.vector.reciprocal(rinv, r_parts[0])
                                nc.vector.tensor_scalar_mul(ep[:, :kx], ep[:, :kx], rinv)
                                nc.sync.dma_start(
                                    out=attnw[ib, h, q0:q0 + 128, 0:kx],
                                    in_=ep[:, :kx].bitcast(mybir.dt.float32),
                                )
                                # transpose P chunks into stage[k, kc, q]
                                n_ch = kx // 128
                                for cg in range((n_ch + 3) // 4):
                                    ng = min(4, n_ch - cg * 4)
                                    pp = pt_ps.tile([128, 512], f32r,
                                                    name=f"pp{ib}{h}{qi}{cg}", tag="pp")
                                    for j in range(ng):
                                        kc = cg * 4 + j
                                        nc.tensor.transpose(
                                            pp[:, j * 128:(j + 1) * 128],
                                            ep[:, kc * 128:(kc + 1) * 128], ident,
                                        )
                                    dst = stage[:, cg * 4:cg * 4 + ng,
                                                iqb * 128:(iqb + 1) * 128]
                                    src = pp.rearrange("p (j q) -> p j q", q=128)[:, :ng, :]
                                    if qi % 2 == 0:
                                        nc.vector.tensor_copy(dst, src)
                                    else:
                                        nc.scalar.copy(dst, src)
                                if causal and kx < kx_sb:
                                    nc.vector.memset(
                                        stage[:, kx // 128:kx_sb // 128,
                                              iqb * 128:(iqb + 1) * 128]
                                        .bitcast(mybir.dt.float32),
                                        0.0,
                                    )
                            # A @ V for this q superblock
                            po = o_ps.tile([128, qsb_sz], mybir.dt.float32,
                                           name=f"po{ib}{h}{iqsb}", tag="po")
                            nkc_sb = kx_sb // 128
                            for kc in range(nkc_sb):
                                nc.tensor.matmul(
                                    po,
                                    v_sb[:, ib * n_kc + kc, :],
                                    stage[:, kc, :],
                                    start=(kc == 0), stop=(kc == nkc_sb - 1),
                                )
                            nc.vector.tensor_copy(
                                or_sb[:, h, ib * s + iqsb * qsb_sz:
                                      ib * s + (iqsb + 1) * qsb_sz],
                                po,
                            )

            # ---- phase E: output projection ----
            with (
                tc.tile_pool(name="f_ps", bufs=2, space="PSUM") as f_ps,
                tc.tile_pool(name="out_pool", bufs=3) as out_pool,
            ):
                for tt in range(t // 128):
                    pf = f_ps.tile([128, hid], mybir.dt.float32,
                                   name=f"pf{tt}", tag="pf")
                    for h in range(HQ):
                        for c in range(hid // 512):
                            nc.tensor.matmul(
                                pf[:, c * 512:(c + 1) * 512],
                                or_sb[:, h, tt * 128:(tt + 1) * 128],
                                wo_sb[:, h, c * 512:(c + 1) * 512],
                                start=(h == 0), stop=(h == HQ - 1),
                            )
                    osb = out_pool.tile([128, hid], f32, name=f"osb{tt}", tag="osb")
                    nc.scalar.copy(osb, pf)
                    nc.sync.dma_start(out=outp[tt * 128:(tt + 1) * 128, :], in_=osb)

    nc.compile()
    return nc


def _get_nc(causal: bool):
    key = ("nc", causal)
    if key not in _CACHE:
        _CACHE[key] = _build_nc(causal)
    return _CACHE[key]


def _is_causal_mask(mask: np.ndarray) -> bool:
    m0 = mask[:, 0]  # [B, S, S]
    tri = np.tril(np.ones((S, S), dtype=bool))
    want = np.where(tri, np.float32(0.0), np.float32(NEG))
    return all(np.array_equal(m0[i], want) for i in range(mask.shape[0]))


def kernel(hidden_states, cos, sin, attention_mask, Wq, bq, Wk, bk, Wv, bv, Wo):
    from concourse.bass_utils import run_bass_kernel_spmd

    hidden_states = np.ascontiguousarray(np.asarray(hidden_states, dtype=np.float32))
    cos = np.asarray(cos, dtype=np.float32)
    sin = np.asarray(sin, dtype=np.float32)
    attention_mask = np.asarray(attention_mask, dtype=np.float32)
    Wq = np.asarray(Wq, dtype=np.float32)
    bq = np.asarray(bq, dtype=np.float32)
    Wk = np.asarray(Wk, dtype=np.float32)
    bk = np.asarray(bk, dtype=np.float32)
    Wv = np.asarray(Wv, dtype=np.float32)
    bv = np.asarray(bv, dtype=np.float32)
    Wo = np.asarray(Wo, dtype=np.float32)

    causal = _is_causal_mask(attention_mask)
    nc = _get_nc(causal)

    hidden_t = np.ascontiguousarray(hidden_states.reshape(T, HID).T)
    cos_t = np.ascontiguousarray(cos.reshape(T, HD).T)
    sinneg_t = np.ascontiguousarray(sin.reshape(T, HD).T)
    sinneg_t[: HD // 2] = -sinneg_t[: HD // 2]

    in_maps = []
    for c in range(N_CORES):
        qs = slice(c * HQ * HD, (c + 1) * HQ * HD)
        kv = c // (N_CORES // NKV)
        kvs = slice(kv * HD, (kv + 1) * HD)
        im = {
            "hidden_t": hidden_t,
            "cos_t": cos_t,
            "sinneg_t": sinneg_t,
            "wq_t": np.ascontiguousarray(Wq[qs].T),
            "wk_t": np.ascontiguousarray(Wk[kvs].T),
            "wv_t": np.ascontiguousarray(Wv[kvs].T),
            "wo_t": np.ascontiguousarray(Wo[:, qs].T),
            "bq": np.ascontiguousarray(bq[qs].reshape(HQ, HD).T),
            "bk": bk[kvs].reshape(HD, 1),
            "bv": bv[kvs].reshape(HD, 1),
        }
        if not causal:
            im["masks"] = np.ascontiguousarray(attention_mask[:, 0]) * np.float32(1.0 / SCALE)
        in_maps.append(im)

    results = run_bass_kernel_spmd(nc, in_maps, core_ids=list(range(N_CORES))).results

    out = np.zeros((T, HID), dtype=np.float32)
    for c in range(N_CORES):
        out += results[c]["outp"]
    out = out.reshape(B, S, HID)

    attn_weights = np.empty((B, NH, S, S), dtype=np.float32)
    for c in range(N_CORES):
        attn_weights[:, c * HQ:(c + 1) * HQ] = results[c]["attnw"]

    return out, attn_weights


def _build_in_maps(inputs):
    """Same host prep as kernel(); returns (causal, in_maps)."""
    hidden_states = np.ascontiguousarray(np.asarray(inputs["hidden_states"], dtype=np.float32))
    cos = np.asarray(inputs["cos"], dtype=np.float32)
    sin = np.asarray(inputs["sin"], dtype=np.float32)
    attention_mask = np.asarray(inputs["attention_mask"], dtype=np.float32)
    Wq = np.asarray(inputs["Wq"], dtype=np.float32)
    bq = np.asarray(inputs["bq"], dtype=np.float32)
    Wk = np.asarray(inputs["Wk"], dtype=np.float32)
    bk = np.asarray(inputs["bk"], dtype=np.float32)
    Wv = np.asarray(inputs["Wv"], dtype=np.float32)
    bv = np.asarray(inputs["bv"], dtype=np.float32)
    Wo = np.asarray(inputs["Wo"], dtype=np.float32)
    causal = _is_causal_mask(attention_mask)
    hidden_t = np.ascontiguousarray(hidden_states.reshape(T, HID).T)
    cos_t = np.ascontiguousarray(cos.reshape(T, HD).T)
    sinneg_t = np.ascontiguousarray(sin.reshape(T, HD).T)
    sinneg_t[: HD // 2] = -sinneg_t[: HD // 2]
    in_maps = []
    for c in range(N_CORES):
        qs = slice(c * HQ * HD, (c + 1) * HQ * HD)
        kv = c // (N_CORES // NKV)
        kvs = slice(kv * HD, (kv + 1) * HD)
        im = {
            "hidden_t": hidden_t,
            "cos_t": cos_t,
            "sinneg_t": sinneg_t,
            "wq_t": np.ascontiguousarray(Wq[qs].T),
            "wk_t": np.ascontiguousarray(Wk[kvs].T),
            "wv_t": np.ascontiguousarray(Wv[kvs].T),
            "wo_t": np.ascontiguousarray(Wo[:, qs].T),
            "bq": np.ascontiguousarray(bq[qs].reshape(HQ, HD).T),
            "bk": bk[kvs].reshape(HD, 1),
            "bv": bv[kvs].reshape(HD, 1),
        }
        if not causal:
            im["masks"] = np.ascontiguousarray(attention_mask[:, 0]) * np.float32(1.0 / SCALE)
        in_maps.append(im)
    return causal, in_maps


def timeline_ns(causal=True):
    """Cost-model (TimelineSim) estimate of per-core HW execution time."""
    from concourse.timeline_sim import TimelineSim

    nc = _get_nc(causal)
    ts = TimelineSim(nc, require_finite=False, require_nnan=False)
    return ts.simulate(), ts


def measure_exec_ns(inputs, reps=12):
    """Marginal wall time per sharded 8-core execution (async pipelined).

    Upper bound on per-core HW time: includes per-dispatch overhead.
    """
    import jax
    import numpy as _np
    from jax.sharding import Mesh, PartitionSpec
    from jax.experimental.shard_map import shard_map
    from concourse.bass2jax import (
        _bass_exec_p,
        install_neuronx_cc_hook,
        partition_id_tensor,
    )
    from concourse import mybir

    causal, in_maps = _build_in_maps(inputs)
    nc = _get_nc(causal)
    install_neuronx_cc_hook()
    partition_name = nc.partition_id_tensor.name if nc.partition_id_tensor else None
    in_names, out_names, out_avals, zero_outs = [], [], [], []
    for alloc in nc.m.functions[0].allocations:
        if not isinstance(alloc, mybir.MemoryLocationSet):
            continue
        name = alloc.memorylocations[0].name
        if alloc.kind == "ExternalInput":
            if name != partition_name:
                in_names.append(name)
        elif alloc.kind == "ExternalOutput":
            shape = tuple(alloc.tensor_shape)
            dtype = mybir.dt.np(alloc.dtype)
            out_names.append(name)
            out_avals.append(jax.core.ShapedArray(shape, dtype))
            zero_outs.append(_np.zeros(shape, dtype))
    n_params = len(in_names)
    in_names_all = in_names + out_names
    if partition_name is not None:
        in_names_all.append(partition_name)

    def _body(*args):
        operands = list(args)
        if partition_name is not None:
            operands.append(partition_id_tensor())
        return tuple(
            _bass_exec_p.bind(
                *operands,
                out_avals=tuple(out_avals),
                in_names=tuple(in_names_all),
                out_names=tuple(out_names),
                lowering_input_output_aliases=(),
                sim_require_finite=True,
                sim_require_nnan=True,
                nc=nc,
            )
        )

    devices = jax.devices()[:N_CORES]
    mesh = Mesh(_np.asarray(devices), ("core",))
    in_specs = (PartitionSpec("core"),) * (n_params + len(out_names))
    out_specs = (PartitionSpec("core"),) * len(out_names)
    fn = jax.jit(
        shard_map(_body, mesh=mesh, in_specs=in_specs, out_specs=out_specs,
                  check_rep=False),
        keep_unused=True,
    )
    per_core = [[_np.asarray(m[n]) for n in in_names] for m in in_maps]
    concat_in = [
        _np.concatenate([per_core[c][i] for c in range(N_CORES)], axis=0)
        for i in range(n_params)
    ]
    concat_zeros = [
        _np.zeros((N_CORES * z.shape[0], *z.shape[1:]), z.dtype) for z in zero_outs
    ]
    dev_args = [jax.device_put(a) for a in concat_in + concat_zeros]
    jax.block_until_ready(fn(*dev_args))

    import time as _time

    def run_n(n):
        best = 1e30
        for _ in range(2):
            t0 = _time.time()
            outs = [fn(*dev_args) for _ in range(n)]
            jax.block_until_ready(outs)
            best = min(best, _time.time() - t0)
        return best

    t_lo = run_n(reps)
    t_hi = run_n(2 * reps)
    return (t_hi - t_lo) / reps * 1e9
